# revision 1
# baseline (speedup 1.0000x reference)
"""CogKR GNN message-passing aggregate (GRU cell over neighbors, masked mean,
scatter back) on 8 Trainium2 NeuronCores, data-parallel over the batch axis.

Math (per batch b, aim-slot t, neighbor m):
  gi = ent[b,t] @ W_ihE^T + rel[r] @ W_ihR^T + b_ih      (384)
  gh = h[n] @ W_hh^T + b_hh                               (384)
  r = sigmoid(gi_r + gh_r); z = sigmoid(gi_z + gh_z)
  n = tanh(gi_n + r * gh_n)
  h_new = (1-z)*n + z*h[n]
  upd[b,t] = sum_valid(h_new) / max(num,1)
  out = node_embeddings with rows aim_nodes[b,t] <- upd[b,t]

Device factorization: precompute row tables A[bt]=ent@W_ihE^T+b_ih,
R[r]=rel_table@W_ihR^T, GH[n]=node@W_hh^T+b_hh on PE, then gather rows per
neighbor position with SWDGE dma_gather (positions on partitions, features on
the free axis), run the GRU nonlinearity on DVE/ACT, and fold the masked mean
into PE matmuls against host-built per-chunk weight selectors.
"""
import numpy as np
import ml_dtypes

_BF = ml_dtypes.bfloat16

try:
    import concourse.bass as bass  # noqa: F401
except ImportError:
    import sys
    sys.path.insert(0, "/opt/trn_rl_repo")

import concourse.bass as bass
import concourse.bacc as bacc
import concourse.mybir as mybir
import concourse.tile as tile
from concourse.bass_utils import run_bass_kernel_spmd

F32 = mybir.dt.float32
BF16 = mybir.dt.bfloat16
I16 = mybir.dt.int16
AF = mybir.ActivationFunctionType

N_CORES = 8
B, TOPK, MN = 128, 32, 64
NODES, H, E = 258, 128, 128
N_REL = 400
BPC = B // N_CORES          # batches per core
NBT = BPC * TOPK            # (b,t) pairs per core



# --- queue-aware DMASW lane assignment -------------------------------------
# Tile round-robins Pool-engine DMA instructions over the 8 DMASW semaphore
# lanes with no knowledge of the SWDGE queue they run on. The per-queue
# reclaim protocol requires each lane to be used by a single queue, so pin
# instructions that carry a queue_num to lanes 2*queue + {0,1}.
import concourse.tile_sem_assignment as _tsa


def _install_queue_aware_lanes():
    if getattr(_tsa, "_queue_lane_patch", False):
        return
    orig = _tsa.TileClockTick._assign_tick

    def _assign_tick(self, inst):
        qn = getattr(inst, "queue_num", None)
        if (
            qn is not None
            and inst.engine == mybir.EngineType.Pool
            and isinstance(inst, _tsa.DMAInst)
        ):
            flips = getattr(self, "_queue_lane_flip", None)
            if flips is None:
                flips = self._queue_lane_flip = {}
            f = flips.get(qn, 0)
            flips[qn] = f ^ 1
            save = self.next_sw_dma_idx
            self.next_sw_dma_idx = (2 * qn + f) % self.swdge_sem_count
            try:
                return orig(self, inst)
            finally:
                self.next_sw_dma_idx = save
        return orig(self, inst)

    _tsa.TileClockTick._assign_tick = _assign_tick
    _tsa._queue_lane_patch = True


_install_queue_aware_lanes()
# ---------------------------------------------------------------------------


def _build_program(c_star: int, repeat: int = 1, mode: str = 'full'):
    S = c_star * 128        # padded positions per batch
    nc = bacc.Bacc("TRN2", target_bir_lowering=False, debug=False, num_devices=1,
                   num_swdge_queues=4)

    node_flat = nc.dram_tensor("node_flat", [BPC * NODES, H], BF16, kind="ExternalInput")
    node_T = nc.dram_tensor("node_T", [BPC, H, NODES], F32, kind="ExternalInput")
    entT = nc.dram_tensor("entT", [E, NBT], F32, kind="ExternalInput")
    relT = nc.dram_tensor("relT", [E, N_REL], F32, kind="ExternalInput")
    wihtE = nc.dram_tensor("wihtE", [E, 3 * H], F32, kind="ExternalInput")
    wihtR = nc.dram_tensor("wihtR", [E, 3 * H], F32, kind="ExternalInput")
    whhT = nc.dram_tensor("whhT", [H, 3 * H], F32, kind="ExternalInput")
    bih = nc.dram_tensor("bih", [1, 3 * H], F32, kind="ExternalInput")
    bhh = nc.dram_tensor("bhh", [1, 3 * H], F32, kind="ExternalInput")
    nidx = nc.dram_tensor("nidx", [128, BPC * S // 16], I16, kind="ExternalInput")
    ridx = nc.dram_tensor("ridx", [128, BPC * S // 16], I16, kind="ExternalInput")
    btidx = nc.dram_tensor("btidx", [128, BPC * S // 16], I16, kind="ExternalInput")
    wsel = nc.dram_tensor("wsel", [BPC, 128, c_star * TOPK], BF16, kind="ExternalInput")
    upd = nc.dram_tensor("upd", [BPC, TOPK, H], F32, kind="ExternalOutput")

    with tile.TileContext(nc) as tc:
        with (
            tc.tile_pool(name="const", bufs=1) as constp,
            tc.tile_pool(name="tab", bufs=2) as tabp,
            tc.tile_pool(name="tps", bufs=2, space="PSUM") as tpsp,
            tc.tile_pool(name="dram", bufs=1, space="DRAM") as dramp,
            tc.tile_pool(name="gbuf", bufs=3) as gbufp,
            tc.tile_pool(name="tmp", bufs=3) as tmpp,
            tc.tile_pool(name="mps", bufs=2, space="PSUM") as mpsp,
        ):
            A_dram = dramp.tile([NBT, 3 * H], BF16)
            R_dram = dramp.tile([N_REL, 3 * H], BF16)
            GH_dram = dramp.tile([BPC * NODES, 4 * H], BF16)

            ones = constp.tile([1, 128], F32)
            nc.vector.memset(ones[:], 1.0)
            wihtE_t = constp.tile([E, 3 * H], F32)
            wihtR_t = constp.tile([E, 3 * H], F32)
            whhT_t = constp.tile([H, 3 * H], F32)
            bih_t = constp.tile([1, 3 * H], F32)
            bhh_t = constp.tile([1, 3 * H], F32)
            entT_t = constp.tile([E, NBT], F32)
            relT_t = constp.tile([E, N_REL], F32)
            nc.sync.dma_start(wihtE_t[:], wihtE[:])
            nc.sync.dma_start(wihtR_t[:], wihtR[:])
            nc.sync.dma_start(whhT_t[:], whhT[:])
            nc.sync.dma_start(bih_t[:], bih[:])
            nc.sync.dma_start(bhh_t[:], bhh[:])
            nc.sync.dma_start(entT_t[:], entT[:])
            nc.sync.dma_start(relT_t[:], relT[:])

            def build_rows(dst_dram, dst_off, lhsT_ap, sz, rhs_ap, bias_ap):
                ps = tpsp.tile([128, 3 * H], F32, tag="tabps")
                nc.tensor.matmul(ps[:sz], lhsT_ap, rhs_ap, start=True, stop=False)
                nc.tensor.matmul(
                    ps[:sz], ones[:, :sz], bias_ap, start=False, stop=True
                )
                sb = tabp.tile([128, 3 * H], BF16, tag="tabsb")
                nc.scalar.copy(sb[:sz], ps[:sz])
                nc.sync.dma_start(dst_dram[dst_off : dst_off + sz, 0 : 3 * H], sb[:sz])

            zerobias = constp.tile([1, 3 * H], F32)
            nc.vector.memset(zerobias[:], 0.0)

            # A table: rows (b,t) of ent @ W_ihE^T + b_ih
            for c in range(NBT // 128):
                build_rows(
                    A_dram, c * 128, entT_t[:, c * 128 : (c + 1) * 128], 128,
                    wihtE_t[:], bih_t[:],
                )
            # R table: rows rel of rel_table @ W_ihR^T (no bias)
            off = 0
            while off < N_REL:
                sz = min(128, N_REL - off)
                build_rows(R_dram, off, relT_t[:, off : off + sz], sz,
                           wihtR_t[:], zerobias[:])
                off += sz
            # GH table: rows (b,node) of node @ W_hh^T + b_hh
            for b in range(BPC):
                ndT = tabp.tile([H, NODES], F32, tag="ndT")
                nc.sync.dma_start(ndT[:], node_T[b, :, :])
                off = 0
                while off < NODES:
                    sz = min(128, NODES - off)
                    build_rows(GH_dram, b * NODES + off, ndT[:, off : off + sz],
                               sz, whhT_t[:], bhh_t[:])
                    off += sz

            # copy raw h into the GHh table tail columns (DRAM->DRAM)
            nc.sync.dma_start(GH_dram[:, 3 * H : 4 * H], node_flat[:])

            # main loop: per batch, gather rows for S positions and run the GRU
            MAXI = 1024  # dma_gather breaks above ~1024 idxs per call

            def gather(out_tile, table_ap, idx_tile, elem, q):
                off = 0
                while off < S:
                    n = min(MAXI, S - off)
                    nc.gpsimd.dma_gather(
                        out_tile[:, off // 128 : (off + n) // 128, :],
                        table_ap,
                        idx_tile[:, off // 16 : (off + n) // 16],
                        n, n, elem,
                        single_packet=False, queue_num=q,
                    )
                    off += n

            iw = S // 16
            for b in [b for _ in range(repeat) for b in range(BPC)]:
                nI = tmpp.tile([128, iw], I16, tag="nI")
                rI = tmpp.tile([128, iw], I16, tag="rI")
                bI = tmpp.tile([128, iw], I16, tag="bI")
                nc.sync.dma_start(nI[:], nidx[:, b * iw : (b + 1) * iw])
                nc.sync.dma_start(rI[:], ridx[:, b * iw : (b + 1) * iw])
                nc.sync.dma_start(bI[:], btidx[:, b * iw : (b + 1) * iw])

                gGH = gbufp.tile([128, c_star, 4 * H], BF16, tag="gGH")
                gR = gbufp.tile([128, c_star, 3 * H], BF16, tag="gR")
                gA = gbufp.tile([128, c_star, 3 * H], BF16, tag="gA")
                if mode != "compute":
                    gather(gGH, GH_dram[:], nI, 4 * H, 0)
                    gather(gR, R_dram[:], rI, 3 * H, 2)
                    gather(gA, A_dram[:], bI, 3 * H, 3)
                else:
                    # tiny gathers keep write-before-read + pipeline deps
                    nc.gpsimd.dma_gather(gGH[:, 0:1, :], GH_dram[:], nI[:, 0:8],
                                         128, 128, 4 * H, single_packet=False)
                    nc.gpsimd.dma_gather(gR[:, 0:1, :], R_dram[:], rI[:, 0:8],
                                         128, 128, 3 * H, single_packet=False)
                    nc.gpsimd.dma_gather(gA[:, 0:1, :], A_dram[:], bI[:, 0:8],
                                         128, 128, 3 * H, single_packet=False)

                u = tmpp.tile([128, c_star, H], BF16, tag="u")
                if mode == "gather":
                    nc.gpsimd.dma_gather(u[:, 0:1, :], node_flat[:], nI[:, 0:8],
                                         128, 128, H, single_packet=False)
                if mode != "gather":
                    # gi = R + A (overwrite gR)
                    nc.vector.tensor_add(gR[:], gR[:], gA[:])
                    rz = tmpp.tile([128, c_star, 2 * H], BF16, tag="rz")
                    nc.vector.tensor_add(
                        rz[:], gR[:, :, 0 : 2 * H], gGH[:, :, 0 : 2 * H]
                    )
                    nc.scalar.activation(rz[:], rz[:], AF.Sigmoid)
                    nc.vector.tensor_mul(u[:], rz[:, :, 0:H], gGH[:, :, 2 * H : 3 * H])
                    nc.vector.tensor_add(u[:], u[:], gR[:, :, 2 * H : 3 * H])
                    nc.scalar.activation(u[:], u[:], AF.Tanh)
                    gh = tmpp.tile([128, c_star, H], BF16, tag="ghd")
                    nc.vector.tensor_sub(gh[:], gGH[:, :, 3 * H : 4 * H], u[:])
                    nc.vector.tensor_mul(gh[:], rz[:, :, H : 2 * H], gh[:])
                    nc.vector.tensor_add(u[:], u[:], gh[:])

                ws = tmpp.tile([128, c_star, TOPK], BF16, tag="ws")
                nc.sync.dma_start(ws[:], wsel[b, :, :])
                ps = mpsp.tile([TOPK, H], F32, tag="mps")
                for c in range(c_star):
                    nc.tensor.matmul(
                        ps[:], ws[:, c, :], u[:, c, :],
                        start=(c == 0), stop=(c == c_star - 1),
                    )
                ub = tmpp.tile([TOPK, H], F32, tag="ub")
                nc.scalar.copy(ub[:], ps[:])
                nc.sync.dma_start(upd[b, :, :], ub[:])

    nc.compile()
    return nc


def _wrap_idx(idx):
    """(BPC, S) int -> (128, BPC*S/16) int16 wrapped/replicated layout."""
    bpc, s = idx.shape
    w = idx.reshape(bpc, s // 16, 16).transpose(0, 2, 1)   # (BPC, 16, S/16)
    w = np.tile(w, (1, 8, 1))                              # (BPC, 128, S/16)
    return np.ascontiguousarray(
        w.transpose(1, 0, 2).reshape(128, bpc * (s // 16))
    ).astype(np.int16)


def _prepare(node_embeddings, entity_table, relation_table, W_ih, W_hh, b_ih,
             b_hh, aim_nodes, aim_entities, neighbors, neighbors_num):
    node_embeddings = np.asarray(node_embeddings, dtype=np.float32)
    entity_table = np.asarray(entity_table, dtype=np.float32)
    relation_table = np.asarray(relation_table, dtype=np.float32)
    W_ih = np.asarray(W_ih, dtype=np.float32)
    W_hh = np.asarray(W_hh, dtype=np.float32)
    b_ih = np.asarray(b_ih, dtype=np.float32)
    b_hh = np.asarray(b_hh, dtype=np.float32)
    aim_nodes_i = np.asarray(aim_nodes).astype(np.int64)
    aim_entities_i = np.asarray(aim_entities).astype(np.int64)
    nb = np.asarray(neighbors).astype(np.int64)
    num = np.asarray(neighbors_num).astype(np.int64)

    denom = (num + (num == 0)).astype(np.float32)
    w_bt = (1.0 / denom).astype(np.float32)

    kb = num.sum(axis=1)
    c_star = max(1, int(np.ceil(kb.max() / 128.0)))
    S = c_star * 128

    ent_rows = entity_table[aim_entities_i]                 # (B, TOPK, E)

    in_maps = []
    for k in range(N_CORES):
        bs = slice(k * BPC, (k + 1) * BPC)
        node_sh = node_embeddings[bs]                       # (BPC, 258, 128)
        node_flat = np.ascontiguousarray(node_sh.reshape(BPC * NODES, H)).astype(_BF)
        node_T = np.ascontiguousarray(node_sh.transpose(0, 2, 1))
        entT = np.ascontiguousarray(
            ent_rows[bs].reshape(NBT, E).T
        )                                                   # (E, NBT)
        relT = np.ascontiguousarray(relation_table.T)       # (E, 400)
        wihtE = np.ascontiguousarray(W_ih[:, :E].T)         # (E, 384)
        wihtR = np.ascontiguousarray(W_ih[:, E:].T)
        whhT = np.ascontiguousarray(W_hh.T)                 # (H, 384)

        nidx = np.zeros((BPC, S), np.int64)
        ridx = np.zeros((BPC, S), np.int64)
        btidx = np.zeros((BPC, S), np.int64)
        wsel = np.zeros((BPC, 128, c_star, TOPK), np.float32)
        for bl in range(BPC):
            b = k * BPC + bl
            t_arr, m_arr = np.nonzero(
                np.arange(MN)[None, :] < num[b][:, None]
            )
            L = len(t_arr)
            if L:
                nidx[bl, :L] = bl * NODES + nb[b, t_arr, m_arr, 0]
                ridx[bl, :L] = nb[b, t_arr, m_arr, 1]
                btidx[bl, :L] = bl * TOPK + t_arr
                pos = np.arange(L)
                wsel[bl, pos % 128, pos // 128, t_arr] = w_bt[b, t_arr]

        in_maps.append({
            "node_flat": node_flat,
            "node_T": node_T,
            "entT": entT,
            "relT": relT,
            "wihtE": wihtE,
            "wihtR": wihtR,
            "whhT": whhT,
            "bih": np.ascontiguousarray(b_ih[None, :]),
            "bhh": np.ascontiguousarray(b_hh[None, :]),
            "nidx": _wrap_idx(nidx),
            "ridx": _wrap_idx(ridx),
            "btidx": _wrap_idx(btidx),
            "wsel": np.ascontiguousarray(
                wsel.reshape(BPC, 128, c_star * TOPK)
            ).astype(_BF),
        })

    return c_star, in_maps, node_embeddings, aim_nodes_i


def kernel(**inputs):
    c_star, in_maps, node_embeddings, aim_nodes_i = _prepare(**inputs)
    nc = _build_program(c_star)
    res = run_bass_kernel_spmd(nc, in_maps, core_ids=list(range(N_CORES)))

    out = node_embeddings.copy()
    bidx = np.arange(B)[:, None]
    upd_full = np.concatenate(
        [res.results[k]["upd"] for k in range(N_CORES)], axis=0
    )                                                       # (B, TOPK, H)
    out[bidx, aim_nodes_i] = upd_full
    return out

